# revision 1
# baseline (speedup 1.0000x reference)
"""NeighborAttentionLayer Trainium2 kernel (8-core data-parallel SPMD).

Strategy
--------
Data-parallel over the batch dim B=64: each of the 8 NeuronCores runs the
full transformer layer for 8 batches (1024 tokens). No collectives.

Host-side prep (numpy, not on HW):
  * weights transposed to [in_features, out_features] and pre-tiled into
    per-tile-contiguous blocks so every weight DMA is one contiguous transfer
  * V projection runs in fp8(e4m3) with DoubleRow perf mode (2 fp8 MACs per
    PE cell): x and v_w are quantized to fp8 host-side; v_w is pre-scaled by
    WS=64 to stay in e4m3's normal range, un-scaled on PSUM eviction
  * q/k projections stay bf16; 1/sqrt(head_dim) folded into the q weights;
    q/k out-features permuted head-pair-interleaved so every head's 320
    features map onto 128-partition tiles as 128+128+64 slices
  * x shard passed natural fp32 (residual), transposed bf16 (q/k matmuls)
    and transposed fp8 (V matmul)

The learned distance-bias MLP adds a per-query bias broadcast over keys;
softmax over keys is invariant to it, so it is skipped. Scores are bounded
(|s| < 9) so softmax runs without max-subtraction. The key-padding mask is
all-ones per the problem spec; a non-trivial mask is applied
multiplicatively on the exp'd scores.

Attention output stays in SBUF (no DRAM roundtrip); FFN2 weights are fully
resident in phase C so LN2+store pipeline per token tile.

Matmuls accumulate in fp32 PSUM. Softmax / layernorm / residuals are fp32.
"""

import numpy as np
import ml_dtypes

# ---- problem constants (hardcoded per contract) ----
B, K, D, H, DFF = 64, 128, 2560, 8, 1024
HD = D // H                    # 320
EPS = 1e-5
NCORES = 8
BL = B // NCORES               # 8 batches per core
TOK = BL * K                   # 1024 tokens per core
P = 128
DT = D // P                    # 20 d-tiles
FT = DFF // P                  # 8 dff-tiles
CH = 512                       # matmul moving-dim chunk (psum bank limit)
NHALF = 2                      # token halves for attention SBUF pressure
THALF = TOK // NHALF           # 512 tokens per half
BHALF = BL // NHALF            # 4 batches per half
QKT = 2 * DT                   # 40 q+k feature tiles
WS = 64.0                      # fp8 weight pre-scale


def _qk_perm():
    """Head-pair interleaved feature order for q (and k) projections."""
    perm = []
    for p in range(H // 2):
        h0, h1 = 2 * p, 2 * p + 1
        perm.extend(range(HD * h0, HD * h0 + 256))         # tiles 5p+0, 5p+1
        perm.extend(range(HD * h0 + 256, HD * h0 + 320))   # tile 5p+2 lo
        perm.extend(range(HD * h1 + 256, HD * h1 + 320))   # tile 5p+2 hi
        perm.extend(range(HD * h1, HD * h1 + 256))         # tiles 5p+3, 5p+4
    return np.array(perm)


def _score_ktiles(h):
    """(tile, row0, row1) triples (within the 20 q-tiles) contracting head h."""
    p = h // 2
    if h % 2 == 0:
        return [(5 * p + 0, 0, 128), (5 * p + 1, 0, 128), (5 * p + 2, 0, 64)]
    return [(5 * p + 3, 0, 128), (5 * p + 4, 0, 128), (5 * p + 2, 64, 128)]


def _ao_segments():
    """Per d-tile (real feature order) segments for attn@V:
    list over tiles of [(head, d0, d1, psum_base), ...]."""
    segs = [[] for _ in range(DT)]
    for h in range(H):
        d = HD * h
        end = HD * (h + 1)
        while d < end:
            nxt = min(end, (d // P + 1) * P)
            segs[d // P].append((h, d, nxt, d % P))
            d = nxt
    return segs


def _tileize(wT, chunk):
    """[Kin, N] -> [N/chunk, 128, Kin/128, chunk] contiguous blocks."""
    kin, n = wT.shape
    ko = kin // P
    return np.ascontiguousarray(
        wT.reshape(ko, P, n // chunk, chunk).transpose(2, 1, 0, 3))


def build_core_program(use_qk_bias, use_v_bias, use_out_bias, use_b1, use_b2,
                       ln1_affine, ln2_affine, use_mask):
    import concourse.bass as bass
    import concourse.bacc as bacc
    import concourse.mybir as mybir
    import concourse.tile as tile
    from concourse.masks import make_identity

    F32 = mybir.dt.float32
    BF16 = mybir.dt.bfloat16
    FP8 = mybir.dt.float8e4
    DR = mybir.MatmulPerfMode.DoubleRow

    nc = bacc.Bacc()
    dp = nc.declare_dram_parameter
    xT = dp("xT", [NHALF, DT, P, THALF], BF16, isOutput=False)
    xT8 = dp("xT8", [NHALF, DT // 2, P, 2, THALF], FP8, isOutput=False)
    x_nat = dp("x", [TOK, D], BF16, isOutput=False)
    qk_wT = dp("qk_wT", [QKT, P, DT, P], BF16, isOutput=False)
    v_wT = dp("v_wT", [D // CH, P, DT, CH], FP8, isOutput=False)
    out_wT = dp("out_wT", [D // CH, P, DT, CH], FP8, isOutput=False)
    w1T = dp("w1T", [FT, P, DT, P], BF16, isOutput=False)
    w2T = dp("w2T", [D // CH, P, FT, CH], BF16, isOutput=False)
    qk_b = dp("qk_b", [2 * D], F32, isOutput=False) if use_qk_bias else None
    v_b = dp("v_b", [D], F32, isOutput=False) if use_v_bias else None
    out_b = dp("out_b", [D], F32, isOutput=False) if use_out_bias else None
    b1 = dp("b1", [DFF], F32, isOutput=False) if use_b1 else None
    b2 = dp("b2", [D], F32, isOutput=False) if use_b2 else None
    ln1_g = dp("ln1_g", [D], F32, isOutput=False) if ln1_affine else None
    ln1_b = dp("ln1_b", [D], F32, isOutput=False) if ln1_affine else None
    ln2_g = dp("ln2_g", [D], F32, isOutput=False) if ln2_affine else None
    ln2_b = dp("ln2_b", [D], F32, isOutput=False) if ln2_affine else None
    mask_in = dp("mask", [BL, K], F32, isOutput=False) if use_mask else None
    out = dp("out", [TOK, D], F32, isOutput=True)

    # x1 kept bf16: FFN1 consumes it bf16 anyway and the bf16 residual
    # costs ~4e-3 rel err; enables XBAR DMA-transpose loads (2-byte only).
    # Two per-half tensors so half-1 stores don't false-conflict (tensor-
    # granular dep tracking) with half-0's transpose-loads.
    x1_dram = [nc.dram_tensor(f"x1_scratch{g}", [THALF, D], BF16)
               for g in range(2)]

    Exp = mybir.ActivationFunctionType.Exp
    Relu = mybir.ActivationFunctionType.Relu
    Sqrt = mybir.ActivationFunctionType.Sqrt
    Square = mybir.ActivationFunctionType.Square
    Copy = mybir.ActivationFunctionType.Copy
    Ident = mybir.ActivationFunctionType.Identity
    AX = mybir.AxisListType.X
    OP = mybir.AluOpType

    def bcast_dram(ap, n_part=P):
        return bass.AP(tensor=ap.tensor, offset=ap.offset,
                       ap=[[0, n_part]] + list(ap.ap))

    ao_segs = _ao_segments()
    KP = DT // 2                 # k-tile pairs for fp8 DoubleRow

    with tile.TileContext(nc) as tc:
        with (
            tc.tile_pool(name="consts", bufs=1) as consts,
        ):
            id_bf = consts.tile([P, P], BF16)
            make_identity(nc, id_bf)
            eps_sb = consts.tile([P, 1], F32)
            nc.vector.memset(eps_sb, EPS)

            # attention output, resident across phases A+B (fp8, 2.62MB);
            # |ao| <~ 2 so e4m3 direct (out_w carries the WS pre-scale)
            aoT_sb = consts.tile([P, BL, DT, P], FP8)

            # out_proj weights, fully resident fp8 (6.55MB); loads issued
            # mid-phase-A (half 1) so they don't block startup DMAs
            wo_all = consts.tile([P, D // CH, DT, CH], FP8)

            qkb_sb = None
            if use_qk_bias:
                qkb_sb = consts.tile([P, QKT], F32)
                nc.sync.dma_start(out=qkb_sb,
                                  in_=qk_b[:].rearrange("(t p) -> p t", p=P))
            vb_sb = None
            if use_v_bias:
                vb_sb = consts.tile([P, D], F32)
                nc.gpsimd.dma_start(out=vb_sb, in_=bcast_dram(v_b[:]))
            outb_sb = None
            if use_out_bias:
                outb_sb = consts.tile([P, D], F32)
                nc.gpsimd.dma_start(out=outb_sb, in_=bcast_dram(out_b[:]))
            b1_sb = None
            if use_b1:
                b1_sb = consts.tile([P, FT], F32)
                nc.sync.dma_start(out=b1_sb,
                                  in_=b1[:].rearrange("(t p) -> p t", p=P))
            b2_sb = None
            if use_b2:
                b2_sb = consts.tile([P, D], F32)
                nc.gpsimd.dma_start(out=b2_sb, in_=bcast_dram(b2[:]))
            ln1g_sb = ln1b_sb = ln2g_sb = ln2b_sb = None
            if ln1_affine:
                ln1g_sb = consts.tile([P, D], F32)
                nc.gpsimd.dma_start(out=ln1g_sb, in_=bcast_dram(ln1_g[:]))
                ln1b_sb = consts.tile([P, D], F32)
                nc.gpsimd.dma_start(out=ln1b_sb, in_=bcast_dram(ln1_b[:]))
            if ln2_affine:
                ln2g_sb = consts.tile([P, D], F32)
                nc.gpsimd.dma_start(out=ln2g_sb, in_=bcast_dram(ln2_g[:]))
                ln2b_sb = consts.tile([P, D], F32)
                nc.gpsimd.dma_start(out=ln2b_sb, in_=bcast_dram(ln2_b[:]))
            mask_sb = None
            if use_mask:
                mask_sb = consts.tile([P, BL, K], F32)
                nc.gpsimd.dma_start(
                    out=mask_sb, in_=bcast_dram(mask_in[:, :]))

            # ======== attention: both halves share one set of buffers ========
            with (
                tc.tile_pool(name="attn_sb", bufs=1) as asb,
                tc.tile_pool(name="awv", bufs=2) as awv,
                tc.tile_pool(name="awq", bufs=3) as awq,
                tc.tile_pool(name="bt", bufs=2) as bt,
            ):
                v_sb = asb.tile([P, BHALF, D], BF16)
                qkT_sb = asb.tile([P, QKT, THALF], BF16)
                # per-ktile input tiles so first matmuls start after ~2 DMAs
                xk8 = [asb.tile([P, 2, THALF], FP8, name=f"xk8_{j}",
                                tag=f"xk8_{j}")
                       for j in range(KP)]
                xk = [asb.tile([P, THALF], BF16, name=f"xk_{k}",
                               tag=f"xk_{k}")
                      for k in range(DT)]

                with (
                    tc.tile_pool(name="aps", bufs=2, space="PSUM") as aps,
                    tc.tile_pool(name="sps", bufs=2, space="PSUM") as sps,
                    tc.tile_pool(name="tps", bufs=2, space="PSUM") as tps,
                    tc.tile_pool(name="ops", bufs=2, space="PSUM") as ops,
                ):
                    def do_v_chunk(c, wv):
                        for t in range(BHALF):
                            ps = aps.tile([P, CH], F32, tag="ps_a")
                            for j in range(KP):
                                nc.tensor.matmul(
                                    ps,
                                    xk8[j][:, :, t * P:(t + 1) * P],
                                    wv[:, 2 * j:2 * j + 2, :],
                                    start=(j == 0), stop=(j == KP - 1),
                                    perf_mode=DR)
                            if use_v_bias:
                                nc.scalar.activation(
                                    out=v_sb[:, t, c * CH:(c + 1) * CH],
                                    in_=ps, func=Ident,
                                    bias=vb_sb[:, c * CH:(c + 1) * CH],
                                    scale=1.0 / WS)
                            else:
                                nc.scalar.activation(
                                    out=v_sb[:, t, c * CH:(c + 1) * CH],
                                    in_=ps, func=Copy, scale=1.0 / WS)

                    for half in range(NHALF):
                        # xk8 first: V chunk 0 runs on 1.3MB of input and
                        # covers PE while the 2.6MB bf16 xk fill lands
                        for j in range(KP):
                            nc.sync.dma_start(out=xk8[j], in_=xT8[half, j])
                        for k in range(DT):
                            nc.sync.dma_start(out=xk[k], in_=xT[half, k])
                        if half == 1:
                            # scalar queue: the ACT stream's half-0 work
                            # naturally delays these past the startup fill
                            for c in range(D // CH):
                                nc.scalar.dma_start(out=wo_all[:, c],
                                                    in_=out_wT[c])

                        for c in range(2):
                            wv = awv.tile([P, DT, CH], FP8, tag="wv")
                            nc.scalar.dma_start(out=wv, in_=v_wT[c])
                            do_v_chunk(c, wv)

                        # Q/K projection (bf16): transposed [feat, tok]
                        for jt in range(QKT):
                            wq = awq.tile([P, DT, P], BF16, tag="wq")
                            nc.scalar.dma_start(out=wq, in_=qk_wT[jt])
                            ps = aps.tile([P, CH], F32, tag="ps_a")
                            for k in range(DT):
                                nc.tensor.matmul(ps, wq[:, k, :], xk[k],
                                                 start=(k == 0),
                                                 stop=(k == DT - 1))
                            if use_qk_bias:
                                nc.scalar.activation(
                                    out=qkT_sb[:, jt, :], in_=ps, func=Ident,
                                    bias=qkb_sb[:, jt:jt + 1], scale=1.0)
                            else:
                                nc.scalar.activation(out=qkT_sb[:, jt, :],
                                                     in_=ps, func=Copy)

                        # remaining V chunks (fp8 DoubleRow)
                        for c in range(2, D // CH):
                            wv = awv.tile([P, DT, CH], FP8, tag="wv")
                            nc.scalar.dma_start(out=wv, in_=v_wT[c])
                            do_v_chunk(c, wv)

                        # attention per batch: scores -> softmax (no
                        # max-shift) -> transposes -> attn@V into aoT_sb
                        for bi in range(BHALF):
                            b = half * BHALF + bi
                            csl = slice(bi * P, (bi + 1) * P)
                            attn = bt.tile([P, H, P], BF16, tag="attn")
                            esum = bt.tile([P, H], F32, tag="esum")
                            rinv = bt.tile([P, H], F32, tag="rinv")
                            attnT = bt.tile([P, H, P], BF16, tag="attnT")
                            for hg in range(H // 4):
                                sc4 = sps.tile([P, 4, P], F32, tag="sc4")
                                for hh in range(4):
                                    h = hg * 4 + hh
                                    kts = _score_ktiles(h)
                                    for i, (t, r0, r1) in enumerate(kts):
                                        nc.tensor.matmul(
                                            sc4[:, hh, :],
                                            qkT_sb[r0:r1, t, csl],
                                            qkT_sb[r0:r1, DT + t, csl],
                                            start=(i == 0),
                                            stop=(i == len(kts) - 1))
                                hsl = slice(hg * 4, hg * 4 + 4)
                                nc.scalar.activation(
                                    out=attn[:, hsl, :], in_=sc4, func=Exp)
                                if use_mask:
                                    for hh in range(4):
                                        h = hg * 4 + hh
                                        nc.vector.tensor_mul(
                                            out=attn[:, h, :],
                                            in0=attn[:, h, :],
                                            in1=mask_sb[:, b, :])
                                nc.vector.tensor_reduce(
                                    out=esum[:, hsl], in_=attn[:, hsl, :],
                                    axis=AX, op=OP.add)
                            nc.vector.reciprocal(out=rinv, in_=esum)
                            for h in range(H):
                                nc.vector.tensor_scalar_mul(
                                    out=attn[:, h, :], in0=attn[:, h, :],
                                    scalar1=rinv[:, h:h + 1])
                            for h in range(H):
                                tp = tps.tile([P, P], BF16, tag="tp")
                                nc.tensor.transpose(tp, attn[:, h, :], id_bf)
                                nc.vector.tensor_copy(out=attnT[:, h, :],
                                                      in_=tp)
                            for tg in range(DT // 4):
                                ao4 = ops.tile([P, 4, P], F32, tag="ao4")
                                for t2 in range(4):
                                    t = tg * 4 + t2
                                    for (h, d0, d1, base) in ao_segs[t]:
                                        w = d1 - d0
                                        nc.tensor.matmul(
                                            ao4[base:base + w, t2, :],
                                            v_sb[:, bi, d0:d1],
                                            attnT[:, h, :],
                                            start=True, stop=True,
                                            tile_position=((0, base) if base
                                                           else None))
                                nc.vector.tensor_copy(
                                    out=aoT_sb[:, b, tg * 4:tg * 4 + 4, :],
                                    in_=ao4)

            # ======== out_proj + residual + LN1 + FFN1, per token group ======
            with tc.tile_pool(name="hres", bufs=1) as hres:
                hT = hres.tile([P, FT, TOK], BF16)
                with (
                    tc.tile_pool(name="csb", bufs=2) as csb,
                    tc.tile_pool(name="cxr", bufs=2) as cxr,
                    tc.tile_pool(name="cy", bufs=2) as cy,
                    tc.tile_pool(name="cx1t", bufs=1) as cx1t,
                    tc.tile_pool(name="dw", bufs=1) as dw,
                    tc.tile_pool(name="cps", bufs=4, space="PSUM") as cps,
                    tc.tile_pool(name="ctps", bufs=2, space="PSUM") as ctps,
                ):
                    # ti-major: out_proj + LN1 per token tile, pipelined
                    # against the next tile's matmuls; x1T comes back from
                    # DRAM via XBAR DMA-transpose (no PE transposes)
                    x1T_h = [cx1t.tile([P, DT, THALF], BF16,
                                       name=f"x1T_h{g}", tag=f"x1T_h{g}")
                             for g in range(2)]

                    def do_tt(tt):
                        # bf16 y: bn_stats accumulates fp32 internally
                        y_t = cy.tile([P, D], BF16, tag="y_t")
                        stats_g = csb.tile([P, 5, 6], F32, tag="stats")
                        # one whole-row residual load per token tile (DMA
                        # instruction count paces the queue, not bytes)
                        xr = cxr.tile([P, D], BF16, tag="xr")
                        nc.sync.dma_start(
                            out=xr, in_=x_nat[tt * P:(tt + 1) * P, :])
                        for c in range(D // CH):
                            ps = cps.tile([P, CH], F32, tag="ps")
                            for j in range(KP):
                                nc.tensor.matmul(
                                    ps,
                                    aoT_sb[:, tt, 2 * j:2 * j + 2, :],
                                    wo_all[:, c, 2 * j:2 * j + 2, :],
                                    start=(j == 0), stop=(j == KP - 1),
                                    perf_mode=DR)
                            if use_out_bias:
                                nc.vector.tensor_add(
                                    out=ps, in0=ps,
                                    in1=outb_sb[:, c * CH:(c + 1) * CH])
                            nc.vector.tensor_add(
                                out=y_t[:, c * CH:(c + 1) * CH],
                                in0=ps, in1=xr[:, c * CH:(c + 1) * CH])
                            nc.vector.bn_stats(
                                out=stats_g[:, c, :],
                                in_=y_t[:, c * CH:(c + 1) * CH])
                        # LN1; x1 -> scratch (residual), x1T via PE
                        mv = csb.tile([P, 2], F32, tag="mv")
                        nc.vector.bn_aggr(out=mv, in_=stats_g)
                        std = csb.tile([P, 1], F32, tag="std")
                        nc.scalar.activation(out=std, in_=mv[:, 1:2],
                                             func=Sqrt, bias=eps_sb,
                                             scale=1.0)
                        rstd = csb.tile([P, 1], F32, tag="rstd")
                        nc.vector.reciprocal(out=rstd, in_=std)
                        x1_t = csb.tile([P, D], BF16, tag="x1t")
                        nc.vector.tensor_scalar(
                            out=x1_t, in0=y_t,
                            scalar1=mv[:, 0:1], scalar2=rstd,
                            op0=OP.subtract, op1=OP.mult)
                        if ln1_affine:
                            nc.vector.tensor_mul(out=x1_t, in0=x1_t,
                                                 in1=ln1g_sb)
                            nc.vector.tensor_add(out=x1_t, in0=x1_t,
                                                 in1=ln1b_sb)
                        tg, tr = tt // (BL // 2), tt % (BL // 2)
                        nc.sync.dma_start(
                            out=x1_dram[tg][tr * P:(tr + 1) * P, :],
                            in_=x1_t)
                        # x1T via PE transposes (HW-verified path)
                        for k in range(DT):
                            tp = ctps.tile([P, P], BF16, tag="tp_c")
                            nc.tensor.transpose(
                                tp, x1_t[:, k * P:(k + 1) * P], id_bf)
                            nc.scalar.activation(
                                out=x1T_h[tg][:, k, tr * P:(tr + 1) * P],
                                in_=tp, func=Copy)

                    w1_tiles = {}

                    def get_w1(ft):
                        # w1 tiles stay resident: both token halves reuse
                        # the same weights, halving FFN1 weight traffic
                        if ft not in w1_tiles:
                            w1_tiles[ft] = dw.tile([P, DT, P], BF16,
                                                   name=f"w1_{ft}",
                                                   tag=f"w1_{ft}")
                            nc.scalar.dma_start(out=w1_tiles[ft],
                                                in_=w1T[ft])
                        return w1_tiles[ft]

                    def do_ffn1(g, fts):
                        osl = slice(g * THALF, (g + 1) * THALF)
                        for ft in fts:
                            w1 = get_w1(ft)
                            ps = cps.tile([P, CH], F32, tag="ps")
                            for k in range(DT):
                                nc.tensor.matmul(
                                    ps, w1[:, k, :], x1T_h[g][:, k, :],
                                    start=(k == 0), stop=(k == DT - 1))
                            if use_b1:
                                nc.scalar.activation(
                                    out=hT[:, ft, osl], in_=ps, func=Relu,
                                    bias=b1_sb[:, ft:ft + 1], scale=1.0)
                            else:
                                nc.scalar.activation(out=hT[:, ft, osl],
                                                     in_=ps, func=Relu)

                    for tt in range(BL):
                        do_tt(tt)
                    do_ffn1(0, range(FT))
                    do_ffn1(1, range(FT))

                # ======== FFN2 + residual + LN2, per token tile ========
                with (
                    tc.tile_pool(name="esb", bufs=2) as esb,
                    tc.tile_pool(name="exr", bufs=6) as exr,
                    tc.tile_pool(name="ew", bufs=1) as ew,
                    tc.tile_pool(name="eps", bufs=4, space="PSUM") as epsp,
                ):
                    # FFN2 weights fully resident (5.24MB)
                    w2c = []
                    for c in range(D // CH):
                        w2t = ew.tile([P, FT, CH], BF16, tag=f"w2_{c}")
                        if c == 0:
                            # per-k split so FFN2's first matmul starts fast
                            for k in range(FT):
                                nc.scalar.dma_start(out=w2t[:, k, :],
                                                    in_=w2T[c, :, k, :])
                        else:
                            nc.scalar.dma_start(out=w2t, in_=w2T[c])
                        w2c.append(w2t)
                    for tt in range(BL):
                        y2 = esb.tile([P, D], F32, tag="y2")
                        stats_e = esb.tile([P, 5, 6], F32, tag="stats_e")
                        xr = exr.tile([P, D], BF16, tag="xr_e")
                        tg, tr = tt // (BL // 2), tt % (BL // 2)
                        nc.scalar.dma_start(
                            out=xr,
                            in_=x1_dram[tg][tr * P:(tr + 1) * P, :])
                        for c in range(D // CH):
                            ps = epsp.tile([P, CH], F32, tag="ps_e")
                            for k in range(FT):
                                nc.tensor.matmul(
                                    ps, hT[:, k, tt * P:(tt + 1) * P],
                                    w2c[c][:, k, :],
                                    start=(k == 0), stop=(k == FT - 1))
                            if use_b2:
                                nc.vector.tensor_add(
                                    out=ps, in0=ps,
                                    in1=b2_sb[:, c * CH:(c + 1) * CH])
                            nc.vector.tensor_add(
                                out=y2[:, c * CH:(c + 1) * CH],
                                in0=ps, in1=xr[:, c * CH:(c + 1) * CH])
                            nc.vector.bn_stats(
                                out=stats_e[:, c, :],
                                in_=y2[:, c * CH:(c + 1) * CH])
                        mv = esb.tile([P, 2], F32, tag="mv_e")
                        nc.vector.bn_aggr(out=mv, in_=stats_e)
                        std = esb.tile([P, 1], F32, tag="std_e")
                        nc.scalar.activation(out=std, in_=mv[:, 1:2],
                                             func=Sqrt, bias=eps_sb,
                                             scale=1.0)
                        rstd = esb.tile([P, 1], F32, tag="rstd_e")
                        nc.vector.reciprocal(out=rstd, in_=std)
                        o_t = esb.tile([P, D], F32, tag="o_t")
                        # chunked LN2 apply + store so the tail pipelines
                        for c in range(D // CH):
                            chs = slice(c * CH, (c + 1) * CH)
                            nc.vector.tensor_scalar(
                                out=o_t[:, chs], in0=y2[:, chs],
                                scalar1=mv[:, 0:1], scalar2=rstd,
                                op0=OP.subtract, op1=OP.mult)
                            if ln2_affine:
                                nc.vector.tensor_mul(
                                    out=o_t[:, chs], in0=o_t[:, chs],
                                    in1=ln2g_sb[:, chs])
                                nc.vector.tensor_add(
                                    out=o_t[:, chs], in0=o_t[:, chs],
                                    in1=ln2b_sb[:, chs])
                            nc.sync.dma_start(
                                out=out[tt * P:(tt + 1) * P, chs],
                                in_=o_t[:, chs])

    nc.compile()
    return nc


def _prep_inputs(x, distances, mask, qkv_w, qkv_b, out_w, out_b,
                 bias_w1, bias_b1, bias_w2, bias_b2,
                 ffn_w1, ffn_b1, ffn_w2, ffn_b2,
                 ln1_g, ln1_b, ln2_g, ln2_b):
    """Host-side shard + weight formatting. Returns (flags, in_maps)."""
    bf16 = ml_dtypes.bfloat16
    fp8 = ml_dtypes.float8_e4m3
    perm = _qk_perm()

    q_w = qkv_w[0:D][perm] * np.float32(1.0 / np.sqrt(HD))
    k_w = qkv_w[D:2 * D][perm]
    v_w = qkv_w[2 * D:3 * D]
    qk_wT = _tileize(np.concatenate([q_w, k_w], axis=0).T.astype(bf16), P)
    v_wT = _tileize(np.clip(v_w.T * np.float32(WS), -240, 240).astype(fp8),
                    CH)
    out_wT = _tileize(np.clip(out_w.T * np.float32(WS), -240, 240).astype(fp8),
                      CH)
    w1T = _tileize(ffn_w1.T.astype(bf16), P)
    w2T = _tileize(ffn_w2.T.astype(bf16), CH)

    qk_b = np.concatenate([qkv_b[0:D][perm] * np.float32(1.0 / np.sqrt(HD)),
                           qkv_b[D:2 * D][perm]]).astype(np.float32)
    v_b = np.ascontiguousarray(qkv_b[2 * D:3 * D]).astype(np.float32)

    flags = dict(
        use_qk_bias=bool(np.any(qk_b != 0)),
        use_v_bias=bool(np.any(v_b != 0)),
        use_out_bias=bool(np.any(out_b != 0)),
        use_b1=bool(np.any(ffn_b1 != 0)),
        use_b2=bool(np.any(ffn_b2 != 0)),
        ln1_affine=not (np.all(ln1_g == 1) and np.all(ln1_b == 0)),
        ln2_affine=not (np.all(ln2_g == 1) and np.all(ln2_b == 0)),
        use_mask=not bool(np.all(mask)),
    )

    shared = {"qk_wT": qk_wT, "v_wT": v_wT, "out_wT": out_wT,
              "w1T": w1T, "w2T": w2T}
    if flags["use_qk_bias"]:
        shared["qk_b"] = qk_b
    if flags["use_v_bias"]:
        shared["v_b"] = v_b
    if flags["use_out_bias"]:
        # out_proj runs WS-prescaled (fp8 weights); LN1 undoes the scale
        shared["out_b"] = (out_b * WS).astype(np.float32)
    if flags["use_b1"]:
        shared["b1"] = ffn_b1.astype(np.float32)
    if flags["use_b2"]:
        shared["b2"] = ffn_b2.astype(np.float32)
    if flags["ln1_affine"]:
        shared["ln1_g"] = ln1_g.astype(np.float32)
        shared["ln1_b"] = ln1_b.astype(np.float32)
    if flags["ln2_affine"]:
        shared["ln2_g"] = ln2_g.astype(np.float32)
        shared["ln2_b"] = ln2_b.astype(np.float32)

    in_maps = []
    for c in range(NCORES):
        xc = np.ascontiguousarray(
            x[c * BL:(c + 1) * BL].reshape(TOK, D)).astype(np.float32)
        xcT = xc.T                        # [D, TOK]
        # residual copy pre-scaled by WS: phase B computes WS*(x + ao@Wo)
        # (fp8 out_w carries WS); LN1 is scale-invariant so no unscale
        # needed; bf16 to halve residual DMA traffic
        xc_res = np.ascontiguousarray((xc * np.float32(WS)).astype(bf16))
        # bf16: [NHALF, DT, P, THALF] blocks, contiguous per (half, ktile)
        blocks = xcT.reshape(DT, P, NHALF, THALF).transpose(2, 0, 1, 3)
        xT_blocks = np.ascontiguousarray(blocks.astype(bf16))
        # fp8: [NHALF, KP, P, 2, THALF] with the DoubleRow k-tile pair
        # interleaved per partition (pair j covers features 2j*128..)
        blocks8 = xcT.reshape(DT // 2, 2, P, NHALF, THALF).transpose(
            3, 0, 2, 1, 4)
        xT8_blocks = np.ascontiguousarray(
            np.clip(blocks8, -240, 240).astype(fp8))
        m = {"x": xc_res, "xT": xT_blocks, "xT8": xT8_blocks, **shared}
        if flags["use_mask"]:
            m["mask"] = mask[c * BL:(c + 1) * BL].astype(np.float32)
        in_maps.append(m)
    return flags, in_maps


def run(trace=False, **inputs):
    """Build + run on 8 cores. Returns (output, BassKernelResults)."""
    from concourse.bass_utils import run_bass_kernel_spmd

    inputs = {k: np.asarray(v) for k, v in inputs.items()}
    flags, in_maps = _prep_inputs(**inputs)
    nc = build_core_program(**flags)
    res = run_bass_kernel_spmd(nc, in_maps, list(range(NCORES)), trace=trace)
    out = np.stack([np.asarray(res.results[c]["out"], dtype=np.float32)
                    for c in range(NCORES)])
    return out.reshape(B, K, D), res


def kernel(**inputs):
    out, _ = run(trace=False, **inputs)
    return out



# revision 2
# speedup vs baseline: 1.0045x; 1.0045x over previous
"""NeighborAttentionLayer Trainium2 kernel v2 (8-core data-parallel SPMD).

Strategy (v2): every big matmul runs fp8 DoubleRow; accuracy held ~1.8e-2
via two/three-term fp8 splits and fp16 (not bf16) for all 2-byte tensors:

  QK proj : 2-term split-x fp8 DR (x_hi@w + x_lo@(w/16)), evict fp16 true
  V proj  : 1-term fp8 DR (optional lo term via V_SPLIT)
  scores / softmax / attn@V: fp16 plain matmuls, no max-subtraction;
            attn transposed via XBAR DMA-transpose (not PE)
  out_proj: 1-term fp8 DR, ao quantized fp8 at x16
  FFN1    : 3-term fp8 DR (x1 split on device + w1 split on host)
  FFN2    : 3-term fp8 DR (h split on device + w2 split on host)
  x1T     : XBAR DMA-transpose, then fp8 hi/lo quant on Act/DVE

Scale bookkeeping (all powers of 2, exact):
  x_hi8 = Q(x), x_lo8 = Q(16(x-x_hi)); qw8 = Q(512 q_w/sqrt(HD)),
  kw8 = Q(64 k_w), *_d16 = Q(w8/16) -> psum_q = 512 q, psum_k = 64 k.
  vw8 = Q(64 v_w) -> psum_v = 64 v; all evicted fp16 at true scale.
  aoT8 = Q(16 ao); wo8 = Q(64 wo) -> psum_o = 1024(ao@wo);
  x_res = fp16(1024(x [+out_b])) -> y16 = 1024 y; LN eps' = 1024^2 eps;
  device rstd = true_rstd/1024 so LN2's plain apply lands true-scale.
  x1 kept at 16x (x1h8/x1l8, FFN1 path) and 1024x (FFN2 residual).
  w1h8 = Q(64 w1), w1hd = Q(w1h8/16), w1ld = Q(64(w1 - w1h8/64));
  psum_1 = 1024 z; h8 = relu(psum/64) = 16h (+fp16 copy for the lo term).
  w2 splits likewise -> psum_2 = 1024 ffn; y2 = 1024(ffn + x1).
  Output stored fp16 (true scale), cast to fp32 on host.
"""

import numpy as np
import ml_dtypes

# ---- problem constants (hardcoded per contract) ----
B, K, D, H, DFF = 64, 128, 2560, 8, 1024
HD = D // H                    # 320
EPS = 1e-5
NCORES = 8
P = 128
BL = B // NCORES               # 8 batches per core
TOK = BL * K                   # 1024 tokens per core
DT = D // P                    # 20 d-tiles
KP = DT // 2                   # 10 DR k-tile pairs over D
FT = DFF // P                  # 8 dff-tiles
FP2 = FT // 2                  # 4 DR pairs over DFF
CH = 512                       # psum chunk (one bank of fp32)
NCH = D // CH                  # 5 output chunks of D
QJT = 2 * DT                   # 40 q+k output tiles
V_SPLIT = False                # add x_lo term to V proj (+43us, -err)

SQ = 512.0                     # q weight pre-scale
SK = 64.0                      # k weight pre-scale
WS = 64.0                      # v/wo/w1/w2 weight pre-scale
XR = 1024.0                    # residual / psum scale
EPS_SC = XR * XR * EPS


def _score_ktiles(hh):
    """(q_slot, k_slot) pairs for local head hh (0/1). All matmuls use
    full 128 partitions: the shared q tail tile (slot 2) pairs with one
    of two half-zeroed k tail tiles (slots 7/8), so the other head's
    rows contract against zeros."""
    if hh == 0:
        return [(0, 5), (1, 6), (2, 7)]
    return [(2, 8), (3, 9), (4, 10)]


def _ao_segments(g):
    """attn@V output segments for 2-head group g across psum tiles
    psA (d-tiles 5g..5g+2) / psB (5g+3..5g+4):
    [(ps_idx, tile_in_ps, hh, d0, d1, base), ...] with global d."""
    segs = []
    for hh in range(2):
        h = 2 * g + hh
        d = HD * h
        end = HD * (h + 1)
        while d < end:
            nxt = min(end, (d // P + 1) * P)
            t = d // P - 5 * g
            ps_idx, ti = (0, t) if t < 3 else (1, t - 3)
            segs.append((ps_idx, ti, hh, d, nxt, d % P))
            d = nxt
    return segs


def build_core_program(use_qk_bias, use_v_bias, use_b1, use_b2,
                       ln1_affine, ln2_affine, use_mask, stop_after=""):
    import concourse.bass as bass
    import concourse.bacc as bacc
    import concourse.mybir as mybir
    import concourse.tile as tile

    F32 = mybir.dt.float32
    F16 = mybir.dt.float16
    FP8 = mybir.dt.float8e4
    DR = mybir.MatmulPerfMode.DoubleRow

    nc = bacc.Bacc()
    dp = nc.declare_dram_parameter
    xT8h = dp("xT8h", [KP, P, 2, TOK], FP8, isOutput=False)
    xT8l = dp("xT8l", [KP, P, 2, TOK], FP8, isOutput=False)
    x_res = dp("x_res", [TOK, D], F16, isOutput=False)
    qk_w8 = dp("qk_w8", [QJT, P, KP, 2, P], FP8, isOutput=False)
    qk_w8d = dp("qk_w8d", [QJT, P, KP, 2, P], FP8, isOutput=False)
    v_w8 = dp("v_w8", [NCH, P, KP, 2, CH], FP8, isOutput=False)
    v_w8d = (dp("v_w8d", [NCH, P, KP, 2, CH], FP8, isOutput=False)
             if V_SPLIT else None)
    wo8 = dp("wo8", [NCH, P, KP, 2, CH], FP8, isOutput=False)
    w1h8 = dp("w1h8", [FT, P, KP, 2, P], FP8, isOutput=False)
    w1hd = dp("w1hd", [FT, P, KP, 2, P], FP8, isOutput=False)
    w1ld = dp("w1ld", [FT, P, KP, 2, P], FP8, isOutput=False)
    w2h8 = dp("w2h8", [NCH, P, FP2, 2, CH], FP8, isOutput=False)
    w2hd = dp("w2hd", [NCH, P, FP2, 2, CH], FP8, isOutput=False)
    w2ld = dp("w2ld", [NCH, P, FP2, 2, CH], FP8, isOutput=False)
    qk_b = dp("qk_b", [2 * D], F32, isOutput=False) if use_qk_bias else None
    v_b = dp("v_b", [D], F32, isOutput=False) if use_v_bias else None
    b1 = dp("b1", [DFF], F32, isOutput=False) if use_b1 else None
    b2 = dp("b2", [D], F32, isOutput=False) if use_b2 else None
    ln1_g = dp("ln1_g", [D], F32, isOutput=False) if ln1_affine else None
    ln1_b16 = dp("ln1_b16", [D], F32, isOutput=False) if ln1_affine else None
    ln1_bXR = dp("ln1_bXR", [D], F32, isOutput=False) if ln1_affine else None
    ln2_g = dp("ln2_g", [D], F32, isOutput=False) if ln2_affine else None
    ln2_b = dp("ln2_b", [D], F32, isOutput=False) if ln2_affine else None
    mask_in = dp("mask", [BL, K], F32, isOutput=False) if use_mask else None
    out = dp("out", [TOK, D], F16, isOutput=True)

    Exp = mybir.ActivationFunctionType.Exp
    Relu = mybir.ActivationFunctionType.Relu
    Sqrt = mybir.ActivationFunctionType.Sqrt
    Copy = mybir.ActivationFunctionType.Copy
    Ident = mybir.ActivationFunctionType.Identity
    AX = mybir.AxisListType.X
    OP = mybir.AluOpType

    def bcast_dram(ap, n_part=P):
        return bass.AP(tensor=ap.tensor, offset=ap.offset,
                       ap=[[0, n_part]] + list(ap.ap))

    with tile.TileContext(nc) as tc:
        with tc.tile_pool(name="consts", bufs=1) as consts:
            eps_sb = consts.tile([P, 1], F32)
            nc.vector.memset(eps_sb, EPS_SC)
            # attention output, fp8 at 16x, resident until out_proj done
            aoT8 = consts.tile([P, DT, TOK], FP8)

            qkb_sb = None
            if use_qk_bias:
                qkb_sb = consts.tile([P, QJT], F32)
                nc.sync.dma_start(out=qkb_sb,
                                  in_=qk_b[:].rearrange("(t p) -> p t", p=P))
            vb_sb = None
            if use_v_bias:
                vb_sb = consts.tile([P, D], F32)
                nc.gpsimd.dma_start(out=vb_sb, in_=bcast_dram(v_b[:]))
            b1_sb = None
            if use_b1:
                b1_sb = consts.tile([P, FT], F32)
                nc.sync.dma_start(out=b1_sb,
                                  in_=b1[:].rearrange("(t p) -> p t", p=P))
            b2_sb = None
            if use_b2:
                b2_sb = consts.tile([P, D], F32)
                nc.gpsimd.dma_start(out=b2_sb, in_=bcast_dram(b2[:]))
            ln1g_sb = ln1b16_sb = ln1bXR_sb = ln2g_sb = ln2b_sb = None
            if ln1_affine:
                ln1g_sb = consts.tile([P, D], F32)
                nc.gpsimd.dma_start(out=ln1g_sb, in_=bcast_dram(ln1_g[:]))
                ln1b16_sb = consts.tile([P, D], F32)
                nc.gpsimd.dma_start(out=ln1b16_sb,
                                    in_=bcast_dram(ln1_b16[:]))
                ln1bXR_sb = consts.tile([P, D], F32)
                nc.gpsimd.dma_start(out=ln1bXR_sb,
                                    in_=bcast_dram(ln1_bXR[:]))
            if ln2_affine:
                ln2g_sb = consts.tile([P, D], F32)
                nc.gpsimd.dma_start(out=ln2g_sb, in_=bcast_dram(ln2_g[:]))
                ln2b_sb = consts.tile([P, D], F32)
                nc.gpsimd.dma_start(out=ln2b_sb, in_=bcast_dram(ln2_b[:]))
            mask_sb = None
            if use_mask:
                mask_sb = consts.tile([P, BL, K], F32)
                nc.gpsimd.dma_start(out=mask_sb,
                                    in_=bcast_dram(mask_in[:, :]))

            # ================= phase A: attention =================
            with (
                tc.tile_pool(name="axk", bufs=1) as axk,
                tc.tile_pool(name="aqk", bufs=2) as aqk,
                tc.tile_pool(name="awv", bufs=2) as awv,
                tc.tile_pool(name="awq", bufs=3) as awq,
                tc.tile_pool(name="att", bufs=4) as att,
                tc.tile_pool(name="aps", bufs=2, space="PSUM") as aps,
                tc.tile_pool(name="sps", bufs=2, space="PSUM") as sps,
                tc.tile_pool(name="ops", bufs=2, space="PSUM") as ops,
            ):
                xkh = [axk.tile([P, 2, TOK], FP8, name=f"xkh{j}",
                                tag=f"xkh{j}") for j in range(KP)]
                xkl = [axk.tile([P, 2, TOK], FP8, name=f"xkl{j}",
                                tag=f"xkl{j}") for j in range(KP)]
                v16 = axk.tile([P, BL, D], F16)
                for j in range(KP):
                    nc.sync.dma_start(out=xkh[j], in_=xT8h[j])
                for j in range(KP):
                    nc.sync.dma_start(out=xkl[j], in_=xT8l[j])

                # ---- V projection ----
                for c in range(NCH):
                    wv = awv.tile([P, KP, 2, CH], FP8, tag="wv")
                    nc.scalar.dma_start(out=wv, in_=v_w8[c])
                    wvd = None
                    if V_SPLIT:
                        wvd = awv.tile([P, KP, 2, CH], FP8, tag="wvd")
                        nc.scalar.dma_start(out=wvd, in_=v_w8d[c])
                    for t in range(BL):
                        ps = aps.tile([P, CH], F32, tag="ps_a")
                        for j in range(KP):
                            nc.tensor.matmul(
                                ps, xkh[j][:, :, t * P:(t + 1) * P],
                                wv[:, j], start=(j == 0),
                                stop=(not V_SPLIT and j == KP - 1),
                                perf_mode=DR)
                        if V_SPLIT:
                            for j in range(KP):
                                nc.tensor.matmul(
                                    ps, xkl[j][:, :, t * P:(t + 1) * P],
                                    wvd[:, j], start=False,
                                    stop=(j == KP - 1), perf_mode=DR)
                        csl = slice(c * CH, (c + 1) * CH)
                        nc.scalar.activation(out=v16[:, t, csl], in_=ps,
                                             func=Copy, scale=1.0 / WS)
                        if use_v_bias:
                            nc.vector.tensor_tensor(
                                out=v16[:, t, csl], in0=v16[:, t, csl],
                                in1=vb_sb[:, csl], op=OP.add)

                # ---- Q/K proj + attention per 2-head group ----
                for g in range(H // 2 if stop_after != "v" else 0):
                    # slots 0-4: q tiles; 5-10: k tiles (tail split 7/8)
                    qkT = aqk.tile([P, 11, TOK], F16, tag="qkT")
                    nc.vector.memset(qkT[64:128, 7, :], 0.0)
                    nc.vector.memset(qkT[0:64, 8, :], 0.0)
                    for i in range(10):
                        jt = (5 * g + i) if i < 5 else (DT + 5 * g + i - 5)
                        wq = awq.tile([P, KP, 2, P], FP8, tag="wq")
                        nc.scalar.dma_start(out=wq, in_=qk_w8[jt])
                        wqd = awq.tile([P, KP, 2, P], FP8, tag="wqd")
                        nc.scalar.dma_start(out=wqd, in_=qk_w8d[jt])
                        sc = SQ if i < 5 else SK
                        for ch2 in range(2):
                            tsl = slice(ch2 * CH, (ch2 + 1) * CH)
                            ps = aps.tile([P, CH], F32, tag="ps_a")
                            for j in range(KP):
                                nc.tensor.matmul(
                                    ps, wq[:, j], xkh[j][:, :, tsl],
                                    start=(j == 0), stop=False,
                                    perf_mode=DR)
                            for j in range(KP):
                                nc.tensor.matmul(
                                    ps, wqd[:, j], xkl[j][:, :, tsl],
                                    start=False, stop=(j == KP - 1),
                                    perf_mode=DR)
                            ki = i - 5
                            if i < 5:
                                outs = [(slice(0, P), i)]
                            elif ki == 2:
                                outs = [(slice(0, 64), 7),
                                        (slice(64, P), 8)]
                            else:
                                outs = [(slice(0, P),
                                         [5, 6, None, 9, 10][ki])]
                            for prng, slot in outs:
                                if use_qk_bias:
                                    nc.scalar.activation(
                                        out=qkT[prng, slot, tsl],
                                        in_=ps[prng, :], func=Ident,
                                        bias=qkb_sb[prng, jt:jt + 1],
                                        scale=1.0 / sc)
                                else:
                                    nc.scalar.activation(
                                        out=qkT[prng, slot, tsl],
                                        in_=ps[prng, :], func=Copy,
                                        scale=1.0 / sc)

                    # scores + softmax (pipelined over batches)
                    attnT = [None] * BL
                    for b in range(BL if stop_after != "qk" else 0):
                        bsl = slice(b * P, (b + 1) * P)
                        sc2 = sps.tile([P, 2, P], F32, tag="sc2")
                        for hh in range(2):
                            kts = _score_ktiles(hh)
                            for i2, (qs, ks) in enumerate(kts):
                                nc.tensor.matmul(
                                    sc2[:, hh, :],
                                    qkT[:, qs, bsl],
                                    qkT[:, ks, bsl],
                                    start=(i2 == 0),
                                    stop=(i2 == len(kts) - 1))
                        a16 = att.tile([P, 2, P], F16, tag="a16")
                        nc.scalar.activation(out=a16, in_=sc2, func=Exp)
                        if use_mask:
                            for hh in range(2):
                                nc.vector.tensor_tensor(
                                    out=a16[:, hh, :], in0=a16[:, hh, :],
                                    in1=mask_sb[:, b, :], op=OP.mult)
                        esum = att.tile([P, 2], F32, tag="esum")
                        nc.vector.tensor_reduce(out=esum, in_=a16,
                                                axis=AX, op=OP.add)
                        rinv = att.tile([P, 2], F32, tag="rinv")
                        nc.vector.reciprocal(out=rinv, in_=esum)
                        for hh in range(2):
                            nc.vector.tensor_scalar_mul(
                                out=a16[:, hh, :], in0=a16[:, hh, :],
                                scalar1=rinv[:, hh:hh + 1])
                        aT = att.tile([P, 2, P], F16, tag="aT")
                        if stop_after != "noxbar":
                            for hh in range(2):
                                nc.sync.dma_start_transpose(
                                    out=aT[:, hh, :], in_=a16[:, hh, :])
                        attnT[b] = aT

                    segs = _ao_segments(g)
                    for b in range(BL if stop_after not in ("qk", "noxbar")
                                   else 0):
                        bsl = slice(b * P, (b + 1) * P)
                        psA = ops.tile([P, 3, P], F32, tag="psA")
                        psB = ops.tile([P, 2, P], F32, tag="psB")
                        pss = (psA, psB)
                        for (pi, ti, hh, d0, d1, base) in segs:
                            w = d1 - d0
                            nc.tensor.matmul(
                                pss[pi][base:base + w, ti, :],
                                v16[:, b, d0:d1], attnT[b][:, hh, :],
                                start=True, stop=True,
                                tile_position=((0, base) if base else None))
                        nc.scalar.activation(
                            out=aoT8[:, 5 * g:5 * g + 3, bsl], in_=psA,
                            func=Copy, scale=16.0)
                        nc.scalar.activation(
                            out=aoT8[:, 5 * g + 3:5 * g + 5, bsl], in_=psB,
                            func=Copy, scale=16.0)

            # ========== phase B..E: out_proj/LN1/FFN1/FFN2/LN2 ==========
            do_b = stop_after not in ("v", "qk", "noxbar", "attn")
            with tc.tile_pool(name="bres", bufs=1) as bres:
                x1_res = bres.tile([P, BL, D], F16)       # 1024*x1

                with tc.tile_pool(name="bq8", bufs=1) as bq8:
                    x1h8 = bq8.tile([P, DT, TOK], FP8)    # 16*x1
                    x1l8 = bq8.tile([P, DT, TOK], FP8)    # 256*res

                    # ---- phase B: out_proj + residual + LN1 + x1 prep ----
                    with (
                        tc.tile_pool(name="bwo", bufs=1) as bwo,
                        tc.tile_pool(name="bxr", bufs=2) as bxr,
                        tc.tile_pool(name="by", bufs=2) as by,
                        tc.tile_pool(name="bst", bufs=2) as bst,
                        tc.tile_pool(name="bxt", bufs=2) as bxt,
                        tc.tile_pool(name="bq", bufs=2) as bq,
                        tc.tile_pool(name="bps", bufs=4, space="PSUM") as bps,
                    ):
                        wo_t = []
                        for c in range(NCH if do_b else 0):
                            wt = bwo.tile([P, KP, 2, CH], FP8, tag=f"wo{c}")
                            nc.scalar.dma_start(out=wt, in_=wo8[c])
                            wo_t.append(wt)
                        for tt in range(BL if do_b else 0):
                            tsl = slice(tt * P, (tt + 1) * P)
                            xr = bxr.tile([P, D], F16, tag="xr")
                            nc.sync.dma_start(out=xr, in_=x_res[tsl, :])
                            y16 = by.tile([P, D], F16, tag="y16")
                            stats = bst.tile([P, NCH, 6], F32, tag="st")
                            for c in range(NCH):
                                csl = slice(c * CH, (c + 1) * CH)
                                ps = bps.tile([P, CH], F32, tag="ps_b")
                                for j in range(KP):
                                    nc.tensor.matmul(
                                        ps, aoT8[:, 2 * j:2 * j + 2, tsl],
                                        wo_t[c][:, j], start=(j == 0),
                                        stop=(j == KP - 1), perf_mode=DR)
                                nc.vector.tensor_tensor(
                                    out=y16[:, csl], in0=ps,
                                    in1=xr[:, csl], op=OP.add)
                                nc.vector.bn_stats(out=stats[:, c, :],
                                                   in_=y16[:, csl])
                            mv = bst.tile([P, 2], F32, tag="mv")
                            nc.vector.bn_aggr(out=mv, in_=stats)
                            std = bst.tile([P, 1], F32, tag="std")
                            nc.scalar.activation(out=std, in_=mv[:, 1:2],
                                                 func=Sqrt, bias=eps_sb,
                                                 scale=1.0)
                            rstd = bst.tile([P, 1], F32, tag="rstd")
                            nc.vector.reciprocal(out=rstd, in_=std)
                            r16 = bst.tile([P, 1], F32, tag="r16")
                            nc.scalar.activation(out=r16, in_=rstd,
                                                 func=Copy, scale=16.0)
                            rXR = bst.tile([P, 1], F32, tag="rXR")
                            nc.scalar.activation(out=rXR, in_=rstd,
                                                 func=Copy, scale=XR)
                            x1t = by.tile([P, D], F16, tag="x1t")
                            nc.vector.tensor_scalar(
                                out=x1t, in0=y16, scalar1=mv[:, 0:1],
                                scalar2=r16, op0=OP.subtract, op1=OP.mult)
                            nc.vector.tensor_scalar(
                                out=x1_res[:, tt, :], in0=y16,
                                scalar1=mv[:, 0:1], scalar2=rXR,
                                op0=OP.subtract, op1=OP.mult)
                            if ln1_affine:
                                nc.vector.tensor_mul(out=x1t, in0=x1t,
                                                     in1=ln1g_sb)
                                nc.vector.tensor_add(out=x1t, in0=x1t,
                                                     in1=ln1b16_sb)
                                nc.vector.tensor_mul(
                                    out=x1_res[:, tt, :],
                                    in0=x1_res[:, tt, :], in1=ln1g_sb)
                                nc.vector.tensor_add(
                                    out=x1_res[:, tt, :],
                                    in0=x1_res[:, tt, :], in1=ln1bXR_sb)
                            # transpose 16*x1 (XBAR), quantize hi/lo fp8
                            x1T16 = bxt.tile([P, DT, P], F16, tag="x1T")
                            for k2 in range(DT if stop_after != "b_noxbar"
                                            else 0):
                                nc.sync.dma_start_transpose(
                                    out=x1T16[:, k2, :],
                                    in_=x1t[:, k2 * P:(k2 + 1) * P])
                            if stop_after == "b_noxbar":
                                continue
                            nc.scalar.activation(out=x1h8[:, :, tsl],
                                                 in_=x1T16, func=Copy)
                            df = bq.tile([P, DT, P], F16, tag="df")
                            nc.vector.tensor_tensor(
                                out=df, in0=x1T16, in1=x1h8[:, :, tsl],
                                op=OP.subtract)
                            nc.scalar.activation(out=x1l8[:, :, tsl],
                                                 in_=df, func=Copy,
                                                 scale=16.0)

                    # ---- FFN1 (3-term fp8 DR) ----
                    do_f1 = do_b and stop_after not in ("b", "b_noxbar")
                    with tc.tile_pool(name="hpool", bufs=1) as hpool:
                        hTh = hpool.tile([P, FT, TOK], FP8)   # 16*h
                        hTl = hpool.tile([P, FT, TOK], FP8)   # 256*res_h
                        _ffn12(nc, tc, mybir, hTh, hTl, x1h8, x1l8, x1_res,
                               w1h8, w1hd, w1ld, w2h8, w2hd, w2ld,
                               b1_sb, b2_sb, ln2g_sb, ln2b_sb, eps_sb, out,
                               use_b1, use_b2, ln2_affine)
    nc.compile()
    return nc


def _ffn12_unused():
    if True:
        if True:
            if True:
                if True:
                    with (
                        tc.tile_pool(name="cw1", bufs=2) as cw1,
                        tc.tile_pool(name="chp", bufs=2) as chp,
                        tc.tile_pool(name="cps", bufs=4,
                                     space="PSUM") as cps,
                    ):
                        for ft in range(FT):
                            wa = cw1.tile([P, KP, 2, P], FP8, tag="wa")
                            nc.scalar.dma_start(out=wa, in_=w1h8[ft])
                            wb = cw1.tile([P, KP, 2, P], FP8, tag="wb")
                            nc.scalar.dma_start(out=wb, in_=w1hd[ft])
                            wc = cw1.tile([P, KP, 2, P], FP8, tag="wc")
                            nc.scalar.dma_start(out=wc, in_=w1ld[ft])
                            for ch2 in range(2):
                                tsl = slice(ch2 * CH, (ch2 + 1) * CH)
                                ps = cps.tile([P, CH], F32, tag="ps_c")
                                for j in range(KP):
                                    nc.tensor.matmul(
                                        ps, wa[:, j],
                                        x1h8[:, 2 * j:2 * j + 2, tsl],
                                        start=(j == 0), stop=False,
                                        perf_mode=DR)
                                for j in range(KP):
                                    nc.tensor.matmul(
                                        ps, wb[:, j],
                                        x1l8[:, 2 * j:2 * j + 2, tsl],
                                        start=False, stop=False,
                                        perf_mode=DR)
                                for j in range(KP):
                                    nc.tensor.matmul(
                                        ps, wc[:, j],
                                        x1h8[:, 2 * j:2 * j + 2, tsl],
                                        start=False, stop=(j == KP - 1),
                                        perf_mode=DR)
                                bkw = (dict(bias=b1_sb[:, ft:ft + 1])
                                       if use_b1 else {})
                                nc.scalar.activation(
                                    out=hTh[:, ft, tsl], in_=ps,
                                    func=Relu, scale=1.0 / WS, **bkw)
                                h16 = chp.tile([P, CH], F16, tag="h16")
                                nc.scalar.activation(
                                    out=h16, in_=ps, func=Relu,
                                    scale=1.0 / WS, **bkw)
                                df = chp.tile([P, CH], F16, tag="dfh")
                                nc.vector.tensor_tensor(
                                    out=df, in0=h16, in1=hTh[:, ft, tsl],
                                    op=OP.subtract)
                                nc.scalar.activation(out=hTl[:, ft, tsl],
                                                     in_=df, func=Copy,
                                                     scale=16.0)

                # ---- FFN2 (3-term fp8 DR) + residual + LN2 ----
                with (
                    tc.tile_pool(name="ew2", bufs=1) as ew2,
                    tc.tile_pool(name="ey", bufs=2) as ey,
                    tc.tile_pool(name="est", bufs=2) as est,
                    tc.tile_pool(name="eps_", bufs=4, space="PSUM") as epsp,
                ):
                    w2_t = []
                    for c in range(NCH):
                        wa = ew2.tile([P, FP2, 2, CH], FP8, tag=f"w2a{c}")
                        nc.scalar.dma_start(out=wa, in_=w2h8[c])
                        wb = ew2.tile([P, FP2, 2, CH], FP8, tag=f"w2b{c}")
                        nc.scalar.dma_start(out=wb, in_=w2hd[c])
                        wc = ew2.tile([P, FP2, 2, CH], FP8, tag=f"w2c{c}")
                        nc.scalar.dma_start(out=wc, in_=w2ld[c])
                        w2_t.append((wa, wb, wc))
                    for tt in range(BL):
                        tsl = slice(tt * P, (tt + 1) * P)
                        y2 = ey.tile([P, D], F16, tag="y2")
                        stats = est.tile([P, NCH, 6], F32, tag="st2")
                        for c in range(NCH):
                            csl = slice(c * CH, (c + 1) * CH)
                            wa, wb, wc = w2_t[c]
                            ps = epsp.tile([P, CH], F32, tag="ps_e")
                            for j in range(FP2):
                                nc.tensor.matmul(
                                    ps, hTh[:, 2 * j:2 * j + 2, tsl],
                                    wa[:, j], start=(j == 0), stop=False,
                                    perf_mode=DR)
                            for j in range(FP2):
                                nc.tensor.matmul(
                                    ps, hTl[:, 2 * j:2 * j + 2, tsl],
                                    wb[:, j], start=False, stop=False,
                                    perf_mode=DR)
                            for j in range(FP2):
                                nc.tensor.matmul(
                                    ps, hTh[:, 2 * j:2 * j + 2, tsl],
                                    wc[:, j], start=False,
                                    stop=(j == FP2 - 1), perf_mode=DR)
                            nc.vector.tensor_tensor(
                                out=y2[:, csl], in0=ps,
                                in1=x1_res[:, tt, csl], op=OP.add)
                            if use_b2:
                                nc.vector.tensor_tensor(
                                    out=y2[:, csl], in0=y2[:, csl],
                                    in1=b2_sb[:, csl], op=OP.add)
                            nc.vector.bn_stats(out=stats[:, c, :],
                                               in_=y2[:, csl])
                        mv = est.tile([P, 2], F32, tag="mv2")
                        nc.vector.bn_aggr(out=mv, in_=stats)
                        std = est.tile([P, 1], F32, tag="std2")
                        nc.scalar.activation(out=std, in_=mv[:, 1:2],
                                             func=Sqrt, bias=eps_sb,
                                             scale=1.0)
                        rstd = est.tile([P, 1], F32, tag="rstd2")
                        nc.vector.reciprocal(out=rstd, in_=std)
                        o16 = ey.tile([P, D], F16, tag="o16")
                        # scaled stats: rstd = true_rstd/1024, so the plain
                        # apply on 1024-scaled y2 yields the true output
                        nc.vector.tensor_scalar(
                            out=o16, in0=y2, scalar1=mv[:, 0:1],
                            scalar2=rstd, op0=OP.subtract, op1=OP.mult)
                        if ln2_affine:
                            nc.vector.tensor_mul(out=o16, in0=o16,
                                                 in1=ln2g_sb)
                            nc.vector.tensor_add(out=o16, in0=o16,
                                                 in1=ln2b_sb)
                        nc.scalar.dma_start(out=out[tsl, :], in_=o16)

    nc.compile()
    return nc
